# revision 24
# baseline (speedup 1.0000x reference)
"""Distributed APPNP (K-hop personalized-pagerank propagation + MLP) for one
TRN2 chip (8 NeuronCores), written against the concourse Bass/Tile stack.

Graph semantics (matches the reference):
    deg  = segment_sum(ones, dst)
    norm = rsqrt(max(deg, 1))
    h = x;  repeat K times: h = 0.9 * (segsum((h*norm)[src], dst) * norm) + 0.1 * x
    out = relu(h @ W1 + b1) @ W2 + b2

Distribution strategy (1D node row-partition, per the sharding hint):
  - core i owns `npc` consecutive (padded) node rows; edges assigned by dst.
  - each core keeps a full replica of the scaled table g = norm*h
    ([N_PAD, 128] bf16 rows) in HBM; per hop it dma_gathers the src rows of
    its in-edges (dst-sorted, split into 4 src-blocks so row indices fit the
    gather's int16 index format), segment-reduces via one-hot matmuls
    (R one-hots built on DVE with batched is_equal against an iota row),
    applies the teleport epilogue, and AllGathers the shards into the next
    hop's table.  The final hop feeds a small MLP computed from the
    DMA-transposed h^T; per-core logits come back transposed [N_CLS, npc].
"""

import math
import os
import sys
from dataclasses import dataclass, field

import numpy as np
import ml_dtypes

for _p in ("/opt/trn_rl_repo", os.path.expanduser("~/.axon_site/_ro/trn_rl_repo")):
    if _p not in sys.path and os.path.isdir(_p):
        sys.path.append(_p)

from concourse import bass, bacc, mybir, tile  # noqa: E402
from concourse import bass_utils  # noqa: E402

F32 = mybir.dt.float32
BF16 = mybir.dt.bfloat16
I16 = mybir.dt.int16
NP_BF16 = ml_dtypes.bfloat16

P = 128

# When True, build_kernel replaces the AllGather with a local shard write —
# used only by offline cost-model timing estimates.
LOCAL_ONLY = False
# Timing-ablation switches (subset of {"rgen", "matmul", "gather"}); results
# are numerically wrong when non-empty — benchmarking only.
ABLATE = set()


@dataclass
class Cfg:
    n_nodes: int = 100000
    n_edges: int = 800000
    d_data: int = 100
    h_dim: int = 256
    n_cls: int = 47
    k_hops: int = 10
    alpha: float = 0.1
    n_cores: int = 8
    call_slots: int = 2560         # gather slots per dma_gather call
    gbufs: int = 3                  # ring depth per block gather stream
    prefetch: int = 2               # gather calls issued ahead of consumption
    swdge_queues: int = 4           # SWDGE queues to spread gather calls over
    r_batch: int = 24               # dstl columns per one-hot DVE instruction
    psum_bufs: int = 4              # PSUM ring depth for window accumulators
    max_block_rows: int = 25088     # <= 32767 for int16 indices
    align_segments: bool = False    # pack (w,b) segments unaligned (fewer calls)
    cf_engine: str = "sync"         # engine issuing the per-window cx/cf loads

    n_pieces: int = 4               # collective/table pieces per hop (= blocks)

    npc: int = field(init=False)
    n_pad: int = field(init=False)
    n_windows: int = field(init=False)
    n_blocks: int = field(init=False)
    block_rows: int = field(init=False)
    piece_windows: list = field(init=False)  # windows per piece (uneven)
    piece_rows: list = field(init=False)     # rows per piece per core

    def __post_init__(self):
        # npc a multiple of P*n_pieces so each piece is a whole window count
        unit = P * self.n_pieces
        self.npc = int(math.ceil(self.n_nodes / (self.n_cores * unit))) * unit
        self.n_pad = self.npc * self.n_cores
        self.n_windows = self.npc // P
        self.n_blocks = self.n_pieces
        # uneven pieces: the LAST piece is small so the tail AllGather (the
        # per-hop serializer: next hop's gathers need every piece) is short
        lastw = max(1, self.n_windows // 10)
        rem = self.n_windows - lastw
        bigs = [rem // (self.n_pieces - 1)] * (self.n_pieces - 1)
        bigs[0] += rem - sum(bigs)
        self.piece_windows = bigs + [lastw]
        assert sum(self.piece_windows) == self.n_windows
        assert min(self.piece_windows) >= 1
        self.piece_rows = [pw * P for pw in self.piece_windows]
        # gather block = piece: all cores' piece-q rows, [c*pr + j] layout
        self.block_rows = max(self.n_cores * pr for pr in self.piece_rows)
        assert self.block_rows <= 32767
        assert self.call_slots % P == 0


@dataclass
class Sched:
    """Uniform (core-independent) schedule.

    Aligned-padding layout: every (window, block) segment is padded to the
    same multiple-of-128 slot count across all cores, so chunk c of block b
    belongs to exactly one (w, b) pair.  touches[w] = list of
    (b, chunk, col) in consumption order.
    """
    lpad: list
    ncalls: list
    touches: dict
    s0: list            # s0[b][w] = start slot of window w inside block b
    nmax: list          # nmax[b][w] = uniform slot count of the segment
    total_cols: int = 0
    idx_cols: int = 0
    blk_off: list = None
    used: list = None   # used[b] = true slot count of block b (<= lpad[b])


def host_preprocess(inputs: dict, cfg: Cfg):
    """Build per-core device input maps + the uniform schedule."""
    x = np.asarray(inputs["x"], np.float32)
    src = np.asarray(inputs["src"], np.int64)
    dst = np.asarray(inputs["dst"], np.int64)
    W1 = np.asarray(inputs["W1"], np.float32)
    b1 = np.asarray(inputs["b1"], np.float32)
    W2 = np.asarray(inputs["W2"], np.float32)
    b2 = np.asarray(inputs["b2"], np.float32)

    N, D = cfg.n_nodes, cfg.d_data
    NPAD, NPC, W = cfg.n_pad, cfg.npc, cfg.n_windows
    NB, BR = cfg.n_blocks, cfg.block_rows
    C = cfg.n_cores

    deg = np.bincount(dst, minlength=N).astype(np.float32)
    norm = (1.0 / np.sqrt(np.maximum(deg, 1.0))).astype(np.float32)
    norm_pad = np.ones(NPAD, np.float32)
    norm_pad[:N] = norm
    x_pad = np.zeros((NPAD, P), np.float32)
    x_pad[:N, :D] = x

    # table row layout is piece-major: piece q holds all cores' piece-q rows
    # as [c*piece_rows[q] + j].  Gather block q is exactly the region the
    # piece-q AllGather delivers.  Pieces are UNEVEN (last one small).
    prq = np.asarray(cfg.piece_rows, np.int64)
    rb = np.concatenate([[0], np.cumsum(prq)])  # row bounds within a shard
    g0 = (norm_pad[:, None] * x_pad).astype(NP_BF16)
    parts = []
    for q in range(NB):
        for c in range(C):
            parts.append(g0[c * NPC + rb[q]:c * NPC + rb[q + 1]])
    g0 = np.ascontiguousarray(np.concatenate(parts, axis=0))

    per_core = []
    for i in range(C):
        lo, hi = i * NPC, (i + 1) * NPC
        m = (dst >= lo) & (dst < hi)
        s, d = src[m], dst[m] - lo
        o = np.argsort(d, kind="stable")
        s, d = s[o], d[o]
        r = s % NPC
        sq = np.searchsorted(rb, r, side="right") - 1
        sidx = (s // NPC) * prq[sq] + (r - rb[sq])
        blocks = []
        for b in range(NB):
            mb = sq == b
            blocks.append((sidx[mb].astype(np.int32),
                           d[mb].astype(np.int32)))
        per_core.append(blocks)

    seg_bounds = []
    for i in range(C):
        per_b = []
        for b in range(NB):
            d = per_core[i][b][1]
            e0 = np.searchsorted(d, np.arange(W) * P, "left")
            e1 = np.searchsorted(d, (np.arange(W) + 1) * P, "left")
            per_b.append((e0, e1))
        seg_bounds.append(per_b)

    sched = Sched(lpad=[], ncalls=[], touches={}, s0=[], nmax=[], used=[])
    for b in range(NB):
        nmax_b, s0_b = [], []
        cum = 0
        for w in range(W):
            mx = max(seg_bounds[i][b][1][w] - seg_bounds[i][b][0][w]
                     for i in range(C))
            if cfg.align_segments:
                mx = int(math.ceil(mx / P)) * P
            s0_b.append(cum)
            nmax_b.append(int(mx))
            cum += int(mx)
        sched.s0.append(s0_b)
        sched.nmax.append(nmax_b)
        lp = int(math.ceil(max(cum, cfg.call_slots) / cfg.call_slots)) \
            * cfg.call_slots
        sched.lpad.append(lp)
        sched.ncalls.append(lp // cfg.call_slots)
        sched.used.append(int(cum))
    sched.blk_off = np.cumsum([0] + sched.lpad).tolist()
    sched.idx_cols = sched.blk_off[-1] // 16

    total_cols = 0
    for w in range(W):
        tl = []
        for b in range(NB):
            n = sched.nmax[b][w]
            if n == 0:
                continue
            s0 = sched.s0[b][w]
            for c in range(s0 // P, (s0 + n - 1) // P + 1):
                tl.append((b, c, total_cols))
                total_cols += 1
        sched.touches[w] = tl
    sched.total_cols = total_cols

    SENT = -512.0
    in_maps = []
    for i in range(C):
        dstl = np.full((P, total_cols), SENT, np.float32)
        gidx_flat = [np.zeros(sched.lpad[b], np.int16) for b in range(NB)]
        for w in range(W):
            for b, c, col in sched.touches[w]:
                e0, e1 = seg_bounds[i][b][0][w], seg_bounds[i][b][1][w]
                L = e1 - e0
                if L == 0:
                    continue
                sl, d = per_core[i][b]
                s0 = sched.s0[b][w]
                # slots of window w inside chunk c (this core has L of them)
                lo = max(s0, c * P)
                hi = min(s0 + L, (c + 1) * P)
                if hi <= lo:
                    continue
                ee = e0 + (lo - s0)
                dstl[lo - c * P:hi - c * P, col] = \
                    np.clip(d[ee:ee + hi - lo] - w * P, -512, 512)
                gidx_flat[b][lo:hi] = sl[ee:ee + hi - lo].astype(np.int16)
        gidx = np.zeros((P, sched.idx_cols), np.int16)
        for b in range(NB):
            cols = slice(sched.blk_off[b] // 16, sched.blk_off[b + 1] // 16)
            wrapped = gidx_flat[b].reshape(-1, 16).T
            gidx[:, cols] = np.tile(wrapped, (8, 1))

        gl = np.arange(i * NPC, (i + 1) * NPC)
        nrm = norm_pad[gl]
        xg = x_pad[gl]
        cx = (cfg.alpha * nrm[:, None] * xg).reshape(W, P, P).transpose(1, 0, 2)
        cf = (cfg.alpha * xg).reshape(W, P, P).transpose(1, 0, 2)
        av = ((1 - cfg.alpha) * nrm * nrm).reshape(W, P).T
        af = ((1 - cfg.alpha) * nrm).reshape(W, P).T

        w1p = np.zeros((P, cfg.h_dim), NP_BF16)
        w1p[:D, :] = W1.astype(NP_BF16)
        b1c = b1.reshape(cfg.h_dim // P, P).T.astype(np.float32)
        ktiles = cfg.h_dim // P
        w2p = np.zeros((P, ktiles * cfg.n_cls), NP_BF16)
        for t in range(ktiles):
            w2p[:, t * cfg.n_cls:(t + 1) * cfg.n_cls] = \
                W2[t * P:(t + 1) * P, :].astype(NP_BF16)
        b2c = np.zeros((P, 1), np.float32)
        b2c[: cfg.n_cls, 0] = b2
        iota = np.tile(np.arange(P, dtype=np.float32)[None, :],
                       (P, 1)).astype(NP_BF16)

        in_maps.append({
            "g0": np.ascontiguousarray(g0),
            "gidx": np.ascontiguousarray(gidx),
            "dstl": np.ascontiguousarray(dstl.astype(NP_BF16)),
            "cx": np.ascontiguousarray(cx.astype(NP_BF16)),
            "cf": np.ascontiguousarray(cf.astype(NP_BF16)),
            "avec": np.ascontiguousarray(av.astype(np.float32)),
            "afvec": np.ascontiguousarray(af.astype(np.float32)),
            "w1": np.ascontiguousarray(w1p),
            "b1c": np.ascontiguousarray(b1c),
            "w2": np.ascontiguousarray(w2p),
            "b2c": np.ascontiguousarray(b2c),
            "iota": np.ascontiguousarray(iota),
        })
    return in_maps, sched


def declare_params(nc, cfg: Cfg, sched: Sched):
    NPAD, W = cfg.n_pad, cfg.n_windows
    ktiles = cfg.h_dim // P
    p = {}
    p["g0"] = nc.dram_tensor("g0", [NPAD, P], BF16, kind="ExternalInput")
    p["gidx"] = nc.dram_tensor("gidx", [P, sched.idx_cols], I16,
                               kind="ExternalInput")
    p["dstl"] = nc.dram_tensor("dstl", [P, sched.total_cols], BF16,
                               kind="ExternalInput")
    p["cx"] = nc.dram_tensor("cx", [P, W, P], BF16, kind="ExternalInput")
    p["cf"] = nc.dram_tensor("cf", [P, W, P], BF16, kind="ExternalInput")
    p["avec"] = nc.dram_tensor("avec", [P, W], F32, kind="ExternalInput")
    p["afvec"] = nc.dram_tensor("afvec", [P, W], F32, kind="ExternalInput")
    p["w1"] = nc.dram_tensor("w1", [P, cfg.h_dim], BF16, kind="ExternalInput")
    p["b1c"] = nc.dram_tensor("b1c", [P, ktiles], F32, kind="ExternalInput")
    p["w2"] = nc.dram_tensor("w2", [P, ktiles * cfg.n_cls], BF16,
                             kind="ExternalInput")
    p["b2c"] = nc.dram_tensor("b2c", [P, 1], F32, kind="ExternalInput")
    p["iota"] = nc.dram_tensor("iota", [P, P], BF16, kind="ExternalInput")
    p["out"] = nc.dram_tensor("out", [cfg.n_cls, cfg.npc], F32,
                              kind="ExternalOutput")
    return p


def build_kernel(nc, tc, prm, cfg: Cfg, sched: Sched):
    W, NB, NPC = cfg.n_windows, cfg.n_blocks, cfg.npc
    K = cfg.k_hops
    CALL = cfg.call_slots
    GRP = CALL // P
    ktiles = cfg.h_dim // P

    with (
        tc.tile_pool(name="static", bufs=1) as st,
        tc.tile_pool(name="gather", bufs=cfg.gbufs) as gp,
        tc.tile_pool(name="rmat", bufs=3) as rp,
        tc.tile_pool(name="epi", bufs=3) as ep,
        tc.tile_pool(name="shard", bufs=1) as shp,
        tc.tile_pool(name="mlp", bufs=1) as mp,
        tc.tile_pool(name="psum", bufs=cfg.psum_bufs, space="PSUM") as pp,
        tc.tile_pool(name="psmlp", bufs=2, space="PSUM") as pmp,
        tc.tile_pool(name="dram", bufs=1, space="DRAM") as dp,
    ):
        gidx_sb = st.tile([P, sched.idx_cols], I16, tag="gidx")
        nc.sync.dma_start(gidx_sb[:], prm["gidx"][:, :])
        dstl_sb = st.tile([P, sched.total_cols], BF16, tag="dstl")
        nc.sync.dma_start(dstl_sb[:], prm["dstl"][:, :])
        av_sb = st.tile([P, W], F32, tag="avec")
        nc.sync.dma_start(av_sb[:], prm["avec"][:, :])
        af_sb = st.tile([P, W], F32, tag="afvec")
        nc.sync.dma_start(af_sb[:], prm["afvec"][:, :])
        iota_sb = st.tile([P, P], BF16, tag="iota")
        nc.sync.dma_start(iota_sb[:], prm["iota"][:, :])
        w1_sb = st.tile([P, cfg.h_dim], BF16, tag="w1")
        nc.sync.dma_start(w1_sb[:], prm["w1"][:, :])
        b1_sb = st.tile([P, ktiles], F32, tag="b1c")
        nc.sync.dma_start(b1_sb[:], prm["b1c"][:, :])
        w2_sb = st.tile([P, ktiles * cfg.n_cls], BF16, tag="w2")
        nc.sync.dma_start(w2_sb[:], prm["w2"][:, :])
        b2_sb = st.tile([P, 1], F32, tag="b2c")
        nc.sync.dma_start(b2_sb[:], prm["b2c"][:, :])

        cx_sb = st.tile([P, W, P], BF16, tag="cxcf")
        nc.sync.dma_start(cx_sb[:], prm["cx" if K > 1 else "cf"][:, :, :])

        # per-(hop, piece) tables: AllGather #q of hop k fills tabs[k][q],
        # and next hop's block-q gathers depend only on that piece.
        brows = [cfg.n_cores * pr for pr in cfg.piece_rows]
        broff = np.cumsum([0] + brows).tolist()
        wend = np.cumsum(cfg.piece_windows).tolist()
        tabs = [[dp.tile([brows[q], P], BF16, tag=f"tab{k}_{q}",
                         name=f"tab{k}_{q}",
                         addr_space="Local" if LOCAL_ONLY else "Shared")
                 for q in range(NB)]
                for k in range(K - 1)]
        bncq = [dp.tile([cfg.piece_rows[q], P], BF16, tag=f"bnc{q}",
                        name=f"bnc{q}")
                for q in range(NB)]
        bounce = dp.tile([NPC, P], BF16, tag="bounce")

        shard_sb = shp.tile([P, W, P], BF16, tag="shard")

        for k in range(K):
            last = k == K - 1
            if last and K > 1:
                # the teleport constant switches from alpha*norm*x to alpha*x
                # on the final hop; swap the resident tile's contents
                nc.sync.dma_start(cx_sb[:], prm["cf"][:, :, :])
            if k == 0 or "statictab" in ABLATE:
                src_aps = [prm["g0"][broff[b]:broff[b + 1], :]
                           for b in range(NB)]
            else:
                src_aps = [tabs[k - 1][b][:, :] for b in range(NB)]

            tiles = [dict() for _ in range(NB)]

            def issue(b, n, k=k, tiles=tiles, src_aps=src_aps):
                if n in tiles[b] or n >= sched.ncalls[b]:
                    return
                if "gather" in ABLATE:
                    if tiles[b]:
                        tiles[b][n] = next(iter(tiles[b].values()))
                        return
                    t = gp.tile([P, GRP, P], BF16, tag=f"g{b}")
                    nc.vector.memset(t[:], 0.0)
                    tiles[b][n] = t
                    return
                t = gp.tile([P, GRP, P], BF16, tag=f"g{b}")
                base = sched.blk_off[b] + n * CALL
                # the final call of a block only gathers the used slots
                # (rounded up to 128), not the full call grid
                rem = sched.used[b] - n * CALL
                nidx = min(CALL, ((rem + P - 1) // P) * P)
                idx_ap = gidx_sb[:, base // 16:(base + nidx) // 16]
                src_ap = src_aps[b]
                nc.gpsimd.dma_gather(
                    t[:, :nidx // P, :], src_ap, idx_ap,
                    num_idxs=nidx, num_idxs_reg=nidx,
                    elem_size=P, elem_step=P,
                    # one DMA packet holds <=64 descriptors (~960 idxs);
                    # larger calls must span multiple packets
                    single_packet=(nidx <= 960),
                    queue_num=(n * cfg.n_blocks + b) % cfg.swdge_queues,
                )
                tiles[b][n] = t

            for b in range(NB):
                issue(b, 0)
                issue(b, 1)

            RB = cfg.r_batch
            nrb = (sched.total_cols + RB - 1) // RB
            rtiles = {}

            def issue_r(n, k=k, rtiles=rtiles):
                if n in rtiles or n >= nrb:
                    return
                if "rgen" in ABLATE and rtiles:
                    rtiles[n] = next(iter(rtiles.values()))
                    return
                c0 = n * RB
                L = min(sched.total_cols, c0 + RB) - c0
                r_t = rp.tile([P, RB * P], BF16, tag="R")
                d_ap = dstl_sb[:, c0:c0 + L].rearrange(
                    "p (m o) -> p m o", o=1).to_broadcast([P, L, P])
                i_ap = iota_sb[:, :].rearrange(
                    "p (o j) -> p o j", o=1).to_broadcast([P, L, P])
                nc.vector.tensor_tensor(
                    out=r_t[:, : L * P].rearrange("p (m j) -> p m j", j=P),
                    in0=i_ap, in1=d_ap, op=mybir.AluOpType.is_equal)
                rtiles[n] = r_t

            for w in range(W):
                wtouch = sched.touches[w]
                for b, c, col in wtouch:
                    for n in range(c // GRP + 1 + cfg.prefetch):
                        issue(b, n)

                psum_w = pp.tile([P, P], F32, tag="agg", space="PSUM")
                first = True
                for ti, (b, c, col) in enumerate(wtouch):
                    issue_r(col // RB)
                    issue_r(col // RB + 1)
                    r_t = rtiles[col // RB]
                    off = col % RB
                    g_t = tiles[b][c // GRP]
                    is_last = ti == len(wtouch) - 1
                    if "matmul" in ABLATE and not (first or is_last):
                        continue
                    nc.tensor.matmul(
                        out=psum_w[:],
                        lhsT=r_t[:, off * P:(off + 1) * P],
                        rhs=g_t[:, c % GRP, :],
                        start=first, stop=is_last)
                    first = False

                a_ap = (af_sb if last else av_sb)[:, w:w + 1]
                t_sb = ep.tile([P, P], F32, tag="eps")
                if first:
                    nc.vector.memset(t_sb[:], 0.0)
                else:
                    nc.scalar.activation(
                        out=t_sb[:], in_=psum_w[:],
                        func=mybir.ActivationFunctionType.Copy, scale=a_ap)
                nc.vector.tensor_tensor(
                    out=shard_sb[:, w, :], in0=t_sb[:], in1=cx_sb[:, w, :],
                    op=mybir.AluOpType.add)

                if not last and (w + 1) in wend:
                    # piece q of this hop's shard is complete: bounce it to
                    # DRAM and AllGather it while later windows compute
                    q = wend.index(w + 1)
                    pw = cfg.piece_windows[q]
                    w0 = wend[q] - pw
                    nc.sync.dma_start(
                        bncq[q][:, :].rearrange("(w p) c -> p w c", p=P),
                        shard_sb[:, w0:w0 + pw, :])
                    if LOCAL_ONLY:
                        # timing-estimate mode: stand in for the AllGather
                        # with a local write (1/8 of the traffic)
                        nc.sync.dma_start(tabs[k][q][0:cfg.piece_rows[q], :],
                                          bncq[q][:, :])
                    else:
                        nc.gpsimd.collective_compute(
                            "AllGather", mybir.AluOpType.bypass,
                            replica_groups=[list(range(cfg.n_cores))],
                            ins=[bncq[q][:, :].opt()],
                            outs=[tabs[k][q][:, :].opt()],
                        )
            if last:
                nc.sync.dma_start(
                    bounce[:, :].rearrange("(w p) c -> p w c", p=P),
                    shard_sb[:, :, :])

        # ---- MLP ----
        ht_sb = mp.tile([P, NPC], BF16, tag="ht")
        nc.sync.dma_start(ht_sb[:], bounce[:, :], transpose=True)

        NCH = 512
        nch = (NPC + NCH - 1) // NCH
        for n in range(nch):
            n0 = n * NCH
            n1 = min(NPC, n0 + NCH)
            L = n1 - n0
            zt_tiles = []
            for t in range(ktiles):
                ps_z = pmp.tile([P, NCH], F32, tag="psz", space="PSUM")
                nc.tensor.matmul(
                    out=ps_z[:, :L],
                    lhsT=w1_sb[:, t * P:(t + 1) * P],
                    rhs=ht_sb[:, n0:n1],
                    start=True, stop=True)
                z_sb = mp.tile([P, NCH], BF16, tag=f"z{t}")
                nc.scalar.activation(
                    out=z_sb[:, :L], in_=ps_z[:, :L],
                    func=mybir.ActivationFunctionType.Relu,
                    bias=b1_sb[:, t:t + 1])
                zt_tiles.append(z_sb)
            ps_o = pmp.tile([cfg.n_cls, NCH], F32, tag="pso", space="PSUM")
            for t in range(ktiles):
                nc.tensor.matmul(
                    out=ps_o[:, :L],
                    lhsT=w2_sb[:, t * cfg.n_cls:(t + 1) * cfg.n_cls],
                    rhs=zt_tiles[t][:, :L],
                    start=(t == 0), stop=(t == ktiles - 1))
            o_sb = mp.tile([cfg.n_cls, NCH], F32, tag="osb")
            nc.scalar.activation(
                out=o_sb[:, :L], in_=ps_o[:, :L],
                func=mybir.ActivationFunctionType.Identity,
                bias=b2_sb[:cfg.n_cls, :])
            nc.sync.dma_start(prm["out"][:, n0:n1], o_sb[:, :L])


def build_program(cfg: Cfg, sched: Sched, debug=False):
    nc = bacc.Bacc("TRN2", target_bir_lowering=False, debug=debug,
                   num_devices=cfg.n_cores,
                   num_swdge_queues=cfg.swdge_queues)
    prm = declare_params(nc, cfg, sched)
    with tile.TileContext(nc) as tc:
        build_kernel(nc, tc, prm, cfg, sched)
    nc.compile()
    return nc


def assemble_output(outs: list, cfg: Cfg) -> np.ndarray:
    full = np.concatenate([np.asarray(o["out"]).T for o in outs], axis=0)
    return np.ascontiguousarray(full[: cfg.n_nodes, :].astype(np.float32))


def run(inputs: dict, trace: bool = False):
    """Returns (full_output [N, n_cls] fp32, exec_time_ns or None)."""
    cfg = Cfg(n_nodes=inputs["x"].shape[0], n_edges=inputs["src"].shape[0],
              d_data=inputs["x"].shape[1], h_dim=inputs["W1"].shape[1],
              n_cls=inputs["W2"].shape[1])
    in_maps, sched = host_preprocess(inputs, cfg)
    nc = build_program(cfg, sched)
    res = bass_utils.run_bass_kernel_spmd(
        nc, in_maps, core_ids=list(range(cfg.n_cores)), trace=trace)
    out = assemble_output(res.results, cfg)
    return out, res.exec_time_ns


def kernel(**inputs) -> np.ndarray:
    out, _ = run(inputs, trace=bool(os.environ.get("APPNP_TRACE")))
    return out

